# revision 32
# baseline (speedup 1.0000x reference)
"""Multi-head dot-product attention (B=2, S=2048, F=1024, H=16, DH=64, O=1024)
as a Bass/Tile kernel on 8 Trainium2 NeuronCores.

Sharding: data-parallel over B (2) x tensor-parallel over H (4 groups of 4
heads) = 8 cores. Each core computes q/k/v projections for its 4 heads,
softmax attention, and a partial output projection; the host sums the 4
partial outputs per batch element and adds the bias.

Attention works in transposed-score space: sT[k, q] = KT_slice.T @ QT (two
heads packed into PE row-groups 0-63/64-127, running concurrently), one exp
on ACT covers both heads, then y'T = V'.T @ PT where V' carries a ones
column so row 64 accumulates the softmax denominator.

The ACT exp stream (128 x [128,1024] exps at ~1.05us) paces the kernel:
 - need-ordered DMA across the three queues (sync: xq_h0+xkv_h0 per chunk;
   gpsimd: xq_h1 then wk; scalar: wq, xkv_h1 stream, wv, wo - all scalar
   triggers BEFORE the first exp so they never stall the exp pipeline);
   wq/wk/wv are host-pre-tiled to [128, nf, hd] for 4KB DMA lines;
 - exps for blocks 0-1 + blocks 2-3 front stream under the projections;
   blocks 0-3 emit two blocks ahead (the parity pt-slot write-after-read
   dependency locks the scheduler's score/y interleave); blocks 4-6 taper
   to one block ahead so the exp stream extends through block 6's y-loop;
 - the tail keeps the PE warm (matmul denominator broadcast for the last
   chunk) and drains the final outputs as sync+scalar half-transfers.
Measured ~2% faster than the previous baseline at equal clock state, with
identical output (rel err 5.03e-4)."""

import numpy as np

import concourse.bass as bass
import concourse.mybir as mybir
import concourse.tile as tile
from concourse import bacc
from concourse.bass_utils import run_bass_kernel_spmd

F32 = mybir.dt.float32
F16 = mybir.dt.float16
AF = mybir.ActivationFunctionType

B, S, F, H, DH, O = 2, 2048, 1024, 16, 64, 1024
NCORES = 8
HPC = 4  # heads per core
CH = 512  # q-chunk width
P = 128


def build_program(s=S, f=F, o=O, hpc=HPC):
    npair = hpc // 2
    nch = s // CH  # q chunks
    nkt = s // P  # k tiles
    nf = f // P  # contraction tiles for projections
    hd = hpc * DH  # stacked head dims per core (256)

    nc = bacc.Bacc("TRN2", target_bir_lowering=False, debug=False, num_devices=NCORES)

    xqT = nc.dram_tensor("xqT", [f, s], F16, kind="ExternalInput")
    xkvT = nc.dram_tensor("xkvT", [f, s], F16, kind="ExternalInput")
    wq = nc.dram_tensor("wq", [P, f // P, hd], F16, kind="ExternalInput")
    wk = nc.dram_tensor("wk", [P, f // P, hd], F16, kind="ExternalInput")
    wv = nc.dram_tensor("wv", [P, f // P, hd], F16, kind="ExternalInput")
    wo = nc.dram_tensor("wo", [hd, o], F16, kind="ExternalInput")
    out = nc.dram_tensor("out", [s, o], F32, kind="ExternalOutput")

    xqT_t = xqT.ap().rearrange("(t p) n -> p t n", p=P)  # [128, nf, s]
    xkvT_t = xkvT.ap().rearrange("(t p) n -> p t n", p=P)
    wq_t = wq.ap()  # [128, nf, hd] host pre-tiled for 4KB DMA lines
    wk_t = wk.ap()
    wv_t = wv.ap()
    wo_t = wo.ap().rearrange("(t p) n -> p t n", p=P)  # [128, hd//128, o]

    with tile.TileContext(nc) as tc:
        with (
            tc.tile_pool(name="weights", bufs=1) as wpool,
            tc.tile_pool(name="xin", bufs=2) as xpool,
            tc.tile_pool(name="xin3", bufs=3) as xpool3,
            tc.tile_pool(name="qkv", bufs=1) as qkvpool,
            tc.tile_pool(name="pt", bufs=1) as ptpool,
            tc.tile_pool(name="norm", bufs=2) as npool,
            tc.tile_pool(name="outsb", bufs=2) as opool,
        ):
            # ---- weights + constants -------------------------------------
            nf2 = nf // 2
            wq_sb = wpool.tile([P, nf, hd], F16, tag="wq")
            wk_sb = wpool.tile([P, nf, hd], F16, tag="wk")
            wv_sb = wpool.tile([P, nf, hd], F16, tag="wv")
            wo_sb = wpool.tile([P, hd // P, o], F16, tag="wo")
            # Need-ordered DMA: aggregate HBM bandwidth (~300GB/s) binds the
            # start, so every queue issues in need order. The scalar ring
            # carries wq + the whole xkv f-half-1 stream + wv/wo, ALL
            # triggered before the first exp (a trigger behind exps would
            # stall the stream); wk rides gpsimd after chunk 0's xq_h1.
            xkv1_tiles = [
                wpool.tile([P, nf2, CH], F16, tag=f"xkv1_{c}", name=f"xkv1_{c}")
                for c in range(nch)
            ]
            nc.scalar.dma_start(wq_sb[:], wq_t)
            nc.scalar.dma_start(wk_sb[:], wk_t)
            nc.scalar.dma_start(xkv1_tiles[0][:], xkvT_t[:, nf2:nf, 0:CH])
            nc.scalar.dma_start(xkv1_tiles[1][:], xkvT_t[:, nf2:nf, CH : 2 * CH])
            nc.scalar.dma_start(wv_sb[:], wv_t)
            nc.scalar.dma_start(xkv1_tiles[2][:], xkvT_t[:, nf2:nf, 2 * CH : 3 * CH])
            nc.scalar.dma_start(xkv1_tiles[3][:], xkvT_t[:, nf2:nf, 3 * CH : 4 * CH])
            nc.scalar.dma_start(wo_sb[:], wo_t)

            def wq_ft(ft):
                return wq_sb[:, ft]

            def wk_ft(ft):
                return wk_sb[:, ft]
            # memset can't write fp16; memset fp32 scratch, cast-copy
            ones_f32 = wpool.tile([P, 4 * P], F32, tag="ones_f32")
            nc.vector.memset(ones_f32[:], 1.0)
            ones_sb = wpool.tile([1, 4 * P], F16, tag="ones")
            nc.vector.tensor_copy(ones_sb[:], ones_f32[0:1, :])

            # ---- storage -------------------------------------------------
            QT = [
                [qkvpool.tile([P, CH], F16, tag=f"QT{p_}_{c}", name=f"QT{p_}_{c}") for c in range(nch)]
                for p_ in range(npair)
            ]
            KT = [
                [qkvpool.tile([P, CH], F16, tag=f"KT{p_}_{c}", name=f"KT{p_}_{c}") for c in range(nch)]
                for p_ in range(npair)
            ]
            # V': per k-tile [128, hpc, DH+1]; last column is ones
            V = [qkvpool.tile([P, hpc, DH + 1], F16, tag=f"V{kt}", name=f"V{kt}") for kt in range(nkt)]
            YT = [
                [qkvpool.tile([P, CH], F16, tag=f"YT{p_}_{c}", name=f"YT{p_}_{c}") for c in range(nch)]
                for p_ in range(npair)
            ]
            for kt in range(nkt):
                nc.vector.tensor_copy(V[kt][:, :, DH], ones_f32[:, 0:hpc])

            blocks = [(c, p_) for c in range(nch) for p_ in range(npair)]
            PT = {}
            DONE = set()
            pools = {}

            def emit_scores(p_, c, kt):
                ps_s = pools["att"].tile([P, 2 * CH], F32, tag="ps_s", name="ps_s")
                nc.tensor.matmul(
                    ps_s[:, 0:CH],
                    KT[p_][kt // 4][0:DH, (kt % 4) * P : (kt % 4 + 1) * P],
                    QT[p_][c][0:DH, :],
                    tile_position=(0, 0),
                )
                nc.tensor.matmul(
                    ps_s[:, CH : 2 * CH],
                    KT[p_][kt // 4][DH : 2 * DH, (kt % 4) * P : (kt % 4 + 1) * P],
                    QT[p_][c][DH : 2 * DH, :],
                    tile_position=(DH, 0),
                )
                return ps_s

            def emit_score_exp(bi, kt):
                c, p_ = blocks[bi]
                ps_s = emit_scores(p_, c, kt)
                tag = (
                    f"pt{bi}_{kt}"
                    if (bi in (2, 3) and kt < 4)
                    else f"pt{bi % 2}_{kt}"
                )
                pt = ptpool.tile([P, 2 * CH], F16, tag=tag, name=tag)
                nc.scalar.activation(pt[:], ps_s[:], AF.Exp)
                PT[(bi, kt)] = pt
                DONE.add((bi, kt))

            ps_att = tc.alloc_tile_pool(name="ps_att", bufs=2, space="PSUM", side="left")
            if True:
                pools["att"] = ps_att

                # ---- projections (blocks 0-1 scores/exp hidden under) -----
                with (
                    tc.tile_pool(name="ps_projqk", bufs=1, space="PSUM", side="right") as ps_projqk,
                    tc.tile_pool(name="ps_projv", bufs=2, space="PSUM", side="right") as ps_projv,
                ):
                    for wu in range(12):
                        ps_wu = ps_projv.tile([P, P], F32, tag="psV", name="ps_wu")
                        nc.tensor.matmul(ps_wu[:], ones_sb[0:1, 0:P], ones_sb[0:1, 0:P])
                    for wu in range(6):
                        ps_wu = ps_projv.tile([P, CH], F32, tag="psV", name="ps_wu2")
                        nc.tensor.matmul(ps_wu[:], ones_sb[0:1, 0:P], ones_sb[0:1, 0:CH])
                    for c in range(nch):
                        xq_half0 = xpool.tile([P, nf2, CH], F16, tag="xq0", name="xq0")
                        xq_half1 = xpool.tile([P, nf2, CH], F16, tag="xq1", name="xq1")
                        xkv_half0 = xpool3.tile([P, nf2, CH], F16, tag="xkv0", name="xkv0")
                        cs = slice(c * CH, (c + 1) * CH)
                        nc.sync.dma_start(xq_half0[:], xqT_t[:, 0:nf2, cs])
                        nc.sync.dma_start(xkv_half0[:], xkvT_t[:, 0:nf2, cs])
                        nc.gpsimd.dma_start(xq_half1[:], xqT_t[:, nf2:nf, cs])

                        def xq_ft(ft, xq_half0=xq_half0, xq_half1=xq_half1):
                            if ft < nf2:
                                return xq_half0[:, ft]
                            return xq_half1[:, ft - nf2]

                        def xkv_ft(ft, c=c, xkv_half0=xkv_half0):
                            if ft < nf2:
                                return xkv_half0[:, ft]
                            return xkv1_tiles[c][:, ft - nf2]

                        for m in range(npair):
                            psQ = ps_projqk.tile([P, CH], F32, tag=f"psQK{m}", name="psQ")
                            for ft in range(nf):
                                nc.tensor.matmul(
                                    psQ[:],
                                    wq_ft(ft)[:, m * P : (m + 1) * P],
                                    xq_ft(ft),
                                    start=(ft == 0),
                                    stop=(ft == nf - 1),
                                )
                            nc.vector.tensor_copy(QT[m][c][:], psQ[:])
                            psK = ps_projqk.tile([P, CH], F32, tag=f"psQK{m}", name="psK")
                            for ft in range(nf):
                                nc.tensor.matmul(
                                    psK[:],
                                    wk_ft(ft)[:, m * P : (m + 1) * P],
                                    xkv_ft(ft),
                                    start=(ft == 0),
                                    stop=(ft == nf - 1),
                                )
                            nc.vector.tensor_copy(KT[m][c][:], psK[:])
                            for kt in range(4 * c, 4 * c + 4):
                                emit_score_exp(m, kt)
                        for bi2 in (2, 3):
                            for kt in range(min(4 * c + 4, 4)):
                                if c >= 1 and (bi2, kt) not in DONE:
                                    emit_score_exp(bi2, kt)
                        for st in range(4):
                            psV = ps_projv.tile([P, CH], F32, tag="psV", name="psV")
                            for ft in range(nf):
                                nc.tensor.matmul(
                                    psV[:, 0:hd],
                                    xkv_ft(ft)[:, st * P : (st + 1) * P],
                                    wv_sb[:, ft, :],
                                    start=(ft == 0),
                                    stop=(ft == nf - 1),
                                )
                            kt = c * 4 + st
                            nc.vector.tensor_copy(
                                V[kt][:, :, 0:DH],
                                psV[:, 0:hd].rearrange("p (h d) -> p h d", h=hpc),
                            )

                # tapered emission: blocks 0-3 two ahead (zero-slack WAR
                # lock on the parity pt slots), block 4 emits (6,0..11),
                # block 5 emits (6,12..15)+(7,0..7), block 6 emits
                # (7,8..15) - the ACT backlog tapers without running dry
                # and the post-stream tail is just block 7's drain.
                def emit_list(bi, kt):
                    if bi <= 3:
                        return [(bi + 2, kt)]
                    if bi == 4:
                        return [(6, kt)] if kt < 12 else []
                    if bi == 5:
                        return [(6, 12 + kt)] if kt < 4 else [(7, kt - 4)]
                    if bi == 6:
                        return [(7, 8 + kt)] if kt < 4 else []
                    if bi == 7:
                        return [(7, 12 + kt)] if kt < 4 else []
                    return []

                pending = []

                def queue_normalize(p_, c, psY):
                    def emit(h01, psY=psY):
                        den_r = npool.tile([1, CH], F32, tag="den", name="den_r")
                        nc.vector.tensor_copy(den_r[:], psY[h01][DH : DH + 1, :])
                        if c == nch - 1 and p_ == npair - 1:
                            # tail: PE-matmul broadcast keeps the HAM clock
                            # gate open so the final outproj runs at 2.4GHz
                            bc_sb = pools["bc"].tile([DH, CH], F32, tag="bc_ps", name="bc_ps")
                            nc.tensor.matmul(bc_sb[:], ones_f32[0:1, 0:DH], den_r[:])
                        else:
                            bc_sb = npool.tile([DH, CH], F32, tag="bc", name="bc_sb")
                            nc.gpsimd.partition_broadcast(bc_sb[:], den_r[:])
                        inv_sb = npool.tile([DH, CH], F32, tag="inv", name="inv_sb")
                        nc.vector.reciprocal_approx_fast(out=inv_sb[:], in_=bc_sb[:])
                        nc.vector.tensor_tensor(
                            YT[p_][c][h01 * DH : (h01 + 1) * DH, :],
                            psY[h01][0:DH, :],
                            inv_sb[:],
                            mybir.AluOpType.mult,
                        )

                    pending.append(lambda: emit(0))
                    pending.append(lambda: emit(1))

                def queue_outproj(c):
                    for st in range(4):
                        qt = c * 4 + st
                        carrier = {}

                        def emit_half(j, st=st, c=c, carrier=carrier):
                            if j == 0:
                                carrier["out_sb"] = opool.tile([P, o], F32, tag="out_sb", name="out_sb")
                            ps_o = pools["o"].tile([P, CH], F32, tag=pools["otag"], name="ps_o")
                            for m in range(hd // P):
                                nc.tensor.matmul(
                                    ps_o[:],
                                    YT[m][c][:, st * P : (st + 1) * P],
                                    wo_sb[:, m, j * CH : (j + 1) * CH],
                                    start=(m == 0),
                                    stop=(m == hd // P - 1),
                                )
                            nc.vector.tensor_copy(
                                carrier["out_sb"][:, j * CH : (j + 1) * CH], ps_o[:]
                            )

                        def emit_dma_half(j, qt=qt, carrier=carrier):
                            # last chunk: halves on sync+scalar right after
                            # each half-copy (both queues idle by then;
                            # gpsimd too slow for the final drain)
                            rows = out.ap()[qt * P : (qt + 1) * P, :]
                            eng = nc.sync if j == 0 else nc.scalar
                            eng.dma_start(
                                rows[:, j * CH : (j + 1) * CH],
                                carrier["out_sb"][:, j * CH : (j + 1) * CH],
                            )

                        def emit_dma(qt=qt, carrier=carrier):
                            rows = out.ap()[qt * P : (qt + 1) * P, :]
                            eng = nc.sync if qt % 2 == 0 else nc.gpsimd
                            eng.dma_start(rows, carrier["out_sb"][:])

                        if qt >= S // P - 4:
                            pending.append(lambda f_=emit_half: f_(0))
                            pending.append(lambda f_=emit_dma_half: f_(0))
                            pending.append(lambda f_=emit_half: f_(1))
                            pending.append(lambda f_=emit_dma_half: f_(1))
                        else:
                            pending.append(lambda f_=emit_half: f_(0))
                            pending.append(lambda f_=emit_half: f_(1))
                            pending.append(emit_dma)

                def run_block(bi, defer_tail=False):
                    c, p_ = blocks[bi]
                    hA, hB = 2 * p_, 2 * p_ + 1
                    for _ in range(2):
                        if pending:
                            pending.pop(0)()
                    psY = [
                        ps_y0pool.tile([DH + 1, CH], F32, tag="psY0", name="psY0"),
                        ps_y1pool.tile([DH + 1, CH], F32, tag="psY1", name="psY1"),
                    ]
                    for kt in range(nkt):
                        pt = PT.pop((bi, kt))
                        nc.tensor.matmul(
                            psY[0][:],
                            V[kt][:, hA, :],
                            pt[:, 0:CH],
                            start=(kt == 0),
                            stop=(kt == nkt - 1),
                        )
                        nc.tensor.matmul(
                            psY[1][:],
                            V[kt][:, hB, :],
                            pt[:, CH : 2 * CH],
                            start=(kt == 0),
                            stop=(kt == nkt - 1),
                        )
                        if pending and (
                            kt % 2 == 1
                            or len(pending) > 6
                            or bi >= len(blocks) - 2
                        ):
                            pending.pop(0)()
                        for tbi, tkt in emit_list(bi, kt):
                            if (tbi, tkt) not in DONE:
                                emit_score_exp(tbi, tkt)
                    if defer_tail:
                        return psY
                    queue_normalize(p_, c, psY)
                    if p_ == npair - 1:
                        queue_outproj(c)

                ps_y0pool = tc.alloc_tile_pool(name="ps_y0", bufs=2, space="PSUM", side="right")
                ps_y1pool = tc.alloc_tile_pool(name="ps_y1", bufs=1, space="PSUM", side="right")
                ps_opool = tc.alloc_tile_pool(name="ps_o", bufs=1, space="PSUM", side="right")
                pools["o"] = ps_opool
                pools["otag"] = "ps_o"
                for bi in range(len(blocks) - 1):
                    run_block(bi)
                psY7 = run_block(len(blocks) - 1, defer_tail=True)

                ps_att.release()  # 4 left banks free for the drain
                ps_drain = tc.alloc_tile_pool(name="ps_drain", bufs=2, space="PSUM", side="left")
                ps_bc = tc.alloc_tile_pool(name="ps_bc", bufs=2, space="PSUM", side="left")
                pools["o"] = ps_drain
                pools["otag"] = "ps_o2"
                pools["bc"] = ps_bc
                queue_normalize(blocks[-1][1], blocks[-1][0], psY7)
                queue_outproj(blocks[-1][0])
                while pending:
                    pending.pop(0)()
                ps_bc.release()
                ps_drain.release()
                ps_opool.release()
                ps_y1pool.release()
                ps_y0pool.release()

    nc.compile()
    return nc


def make_in_maps(inputs_q, inputs_kv, wq, wk, wv, wo):
    """Shard full inputs into 8 per-core input dicts (host-side)."""
    in_maps = []
    scale = 1.0 / np.sqrt(DH)
    nf, hd = F // P, HPC * DH

    def ftile(w):  # [F, hd] -> [128, nf, hd] (f-tiled for 4KB DMA lines)
        return np.ascontiguousarray(
            w.reshape(nf, P, hd).transpose(1, 0, 2)
        ).astype(np.float16)

    for core in range(NCORES):
        b = core // (NCORES // B)
        hg = core % (NCORES // B)
        hs = slice(hg * HPC, (hg + 1) * HPC)
        in_maps.append(
            {
                "xqT": np.ascontiguousarray(inputs_q[b].T).astype(np.float16),
                "xkvT": np.ascontiguousarray(inputs_kv[b].T).astype(np.float16),
                "wq": ftile((wq[:, hs, :] * scale).reshape(F, hd)),
                "wk": ftile(wk[:, hs, :].reshape(F, hd)),
                "wv": ftile(wv[:, hs, :].reshape(F, hd)),
                "wo": np.ascontiguousarray(wo[hs].reshape(HPC * DH, O)).astype(
                    np.float16
                ),
            }
        )
    return in_maps


_CACHE = {}


def _get_program():
    if "nc" not in _CACHE:
        _CACHE["nc"] = build_program()
    return _CACHE["nc"]


def run_sharded(inputs_q, inputs_kv, wq, wk, wv, wo, bo, **spmd_kwargs):
    """Build in_maps, run on 8 cores, reduce partials. Returns (out, results)."""
    nc = _get_program()
    in_maps = make_in_maps(inputs_q, inputs_kv, wq, wk, wv, wo)
    res = run_bass_kernel_spmd(nc, in_maps, core_ids=list(range(NCORES)), **spmd_kwargs)
    gpb = NCORES // B  # head-group cores per batch element
    out = np.zeros((B, S, O), dtype=np.float32)
    for core in range(NCORES):
        out[core // gpb] += res.results[core]["out"]
    out += np.asarray(bo, dtype=np.float32)
    return out, res


def kernel(inputs_q, inputs_kv, wq, wk, wv, wo, bo):
    out, _ = run_sharded(
        np.asarray(inputs_q),
        np.asarray(inputs_kv),
        np.asarray(wq),
        np.asarray(wk),
        np.asarray(wv),
        np.asarray(wo),
        np.asarray(bo),
    )
    return out
